# revision 26
# baseline (speedup 1.0000x reference)
"""Instruction-minimal Trainium2 kernel for nn_DevignModel (v2).

Same math as kernel.py, but the per-(dst,etype)-slot aggregation is done with
hardware scatter-adds instead of one-hot matmuls:
  per step: 1 dma_gather (all edges' source rows, wave-sorted) ->
  16 dma_scatter_add waves (conflict-free within each wave; 2 halves so slot
  ids fit int16; padded lanes land in a trash slot) into A[slots,128] fp16 in
  DRAM -> 1 DMA-transpose per half to get A^T in SBUF -> per-etype transform
  matmuls + GRU (unchanged) -> h' rows via DMA-transpose -> AllGather.

This targets the measured ~60us/instruction dispatch floor of this system:
~230 instructions/step instead of ~640.
"""
import os
import sys
from contextlib import ExitStack

import numpy as np

sys.path.insert(0, "/opt/trn_rl_repo")
sys.path.insert(0, "/opt/trn_rl_repo/concourse")

import concourse.bacc as bacc
import concourse.bass as bass
import concourse.mybir as mybir
import concourse.tile as tile
from concourse.bass_utils import run_bass_kernel_spmd

F16 = mybir.dt.float16
F32 = mybir.dt.float32
I16 = mybir.dt.int16
AF = mybir.ActivationFunctionType
ALU = mybir.AluOpType

B, N, D = 32, 1024, 128
NN = B * N
E = 262144
T = 8
STEPS = int(os.environ.get("DEVIGN_STEPS", "6"))
READOUT = os.environ.get("DEVIGN_READOUT", "1") == "1"
REPEAT = int(os.environ.get("DEVIGN_REPEAT", "1"))
NOCOLL = os.environ.get("DEVIGN_NOCOLL", "0") == "1"
NOGS = os.environ.get("DEVIGN_NOGS", "0") == "1"      # skip gather+scatter (timing only)
NOMM = os.environ.get("DEVIGN_NOMM", "0") == "1"      # skip transform+GRU matmul block
NOZERO = os.environ.get("DEVIGN_NOZERO", "0") == "1"  # skip A zero-fill
NOTR = os.environ.get("DEVIGN_NOTR", "0") == "1"      # skip A^T DMA transposes
NOROWS = os.environ.get("DEVIGN_NOROWS", "0") == "1"  # skip h'-row transposes
MMSG = int(os.environ.get("DEVIGN_MMSG", "4"))        # sgl iters per half (4=full)

C = 8
NSH = NN // C            # 4096
SLOTS = NSH * T          # 32768
HSLOT = SLOTS // 2       # 16384 slots per half
AROWS = HSLOT + 128      # A rows per half (+ trash region)
TRASH = HSLOT            # trash slot id (local to half)
SG = 512
NSG = NSH // SG
GPC = NSH // N
L1 = N - 2
P1 = (L1 - 3) // 2 + 1
P2 = (P1 - 2) // 2 + 1


# ----------------------------------------------------------------- host prep
def _preprocess(src, dst, etype):
    """Wave-sorted edge lists. Returns (regions, gidx, sidx):
    regions: list of (half, row0, nrows) per scatter instruction, 128-row
             granular in the gather tile; shared across cores.
    gidx[c]: [TOT] int16 gather indices (global node ids; pads = 0)
    sidx[c]: [TOT] int16 scatter indices (half-local slot; pads = TRASH)
    """
    per_core = []
    for c in range(C):
        m = (dst // NSH) == c
        e_src = src[m].astype(np.int64)
        e_slot = (dst[m] - c * NSH).astype(np.int64) * T + etype[m]
        order = np.argsort(e_slot, kind="stable")
        es, sl = e_src[order], e_slot[order]
        first = np.searchsorted(sl, sl)
        rank = np.arange(sl.size) - first
        half = (sl >= HSLOT).astype(np.int64)
        key = rank * 2 + half
        worder = np.lexsort((sl, key))
        per_core.append((es[worder], sl[worder], key[worder]))

    nkey = max(int(k[-1]) for _, _, k in per_core) + 1
    sizes = np.zeros(nkey, dtype=np.int64)
    for _, _, k in per_core:
        cnt = np.bincount(k, minlength=nkey)
        sizes = np.maximum(sizes, cnt)
    padded = ((sizes + 127) // 128) * 128

    regions = []
    row = 0
    for w in range(nkey):
        nr = int(padded[w]) // 128
        if nr == 0:
            continue
        regions.append((int(w % 2), row, nr))
        row += nr
    TOT = row * 128

    gidx = np.zeros((C, TOT), dtype=np.int16)
    sidx = np.full((C, TOT), TRASH, dtype=np.int16)
    for c, (es, sl, k) in enumerate(per_core):
        cnt = np.bincount(k, minlength=nkey)
        pos = 0
        row = 0
        for w in range(nkey):
            nr = int(padded[w]) // 128
            if nr == 0:
                continue
            n = int(cnt[w])
            o = row * 128
            gidx[c, o:o + n] = es[pos:pos + n]
            sidx[c, o:o + n] = (sl[pos:pos + n] % HSLOT)
            pos += n
            row += nr
    return regions, TOT, gidx, sidx


def _wrap_idxs(idx):
    n = idx.shape[0]
    w = idx.astype(np.int16).reshape(n // 16, 16).T
    return np.tile(w, (8, 1))


# --------------------------------------------------------------- device build
def _build(regions, TOT):
    nc = bacc.Bacc("TRN2", target_bir_lowering=False, debug=False, num_devices=C)

    h16_d = nc.dram_tensor("h16", [NN, D], F16, kind="ExternalInput")
    hT0_d = nc.dram_tensor("hT0", [128, NSH], F16, kind="ExternalInput")
    gidx_d = nc.dram_tensor("gidx", [128, TOT // 16], I16, kind="ExternalInput")
    sidx_d = nc.dram_tensor("sidx", [128, TOT // 16], I16, kind="ExternalInput")
    zeros_d = nc.dram_tensor("zeros", [AROWS, D], F16, kind="ExternalInput")
    wmsgT_d = nc.dram_tensor("wmsgT", [128, T * 128], F16, kind="ExternalInput")
    gruW_d = nc.dram_tensor("gruW", [128, 6 * 128], F16, kind="ExternalInput")
    gbias_d = nc.dram_tensor("gbias", [128, 4], F32, kind="ExternalInput")
    bdegT_d = nc.dram_tensor("bdegT", [128, NSH], F16, kind="ExternalInput")
    ident_d = nc.dram_tensor("ident", [128, 128], F16, kind="ExternalInput")
    c1w_d = nc.dram_tensor("c1w", [128, 3 * 128], F16, kind="ExternalInput")
    c2w_d = nc.dram_tensor("c2w", [128, 128], F16, kind="ExternalInput")
    cc1w_d = nc.dram_tensor("cc1w", [128, 12 * 128], F16, kind="ExternalInput")
    cc2w_d = nc.dram_tensor("cc2w", [128, 4 * 128], F16, kind="ExternalInput")
    cbias_d = nc.dram_tensor("cbias", [128, 6], F32, kind="ExternalInput")
    mlp_d = nc.dram_tensor("mlp", [128, 3], F16, kind="ExternalInput")
    mlpb_d = nc.dram_tensor("mlpb", [1, 2], F32, kind="ExternalInput")

    out_d = nc.dram_tensor("out", [1, GPC], F32, kind="ExternalOutput")
    hdbg_d = nc.dram_tensor("hdbg", [128, NSH], F16, kind="ExternalOutput")

    with tile.TileContext(nc) as tc, ExitStack() as ctx:
        sb1 = ctx.enter_context(tc.tile_pool(name="sb1", bufs=1))
        sbw = ctx.enter_context(tc.tile_pool(name="sbw", bufs=2))
        ps = ctx.enter_context(tc.tile_pool(name="ps", bufs=1, space="PSUM"))
        dram = ctx.enter_context(tc.tile_pool(name="dram", bufs=1, space="DRAM"))

        gidx_t = sb1.tile([128, TOT // 16], I16)
        nc.sync.dma_start(gidx_t[:], gidx_d[:])
        sidx_t = sb1.tile([128, TOT // 16], I16)
        nc.sync.dma_start(sidx_t[:], sidx_d[:])
        wmsg_t = sb1.tile([128, T * 128], F16)
        nc.sync.dma_start(wmsg_t[:], wmsgT_d[:])
        gruW_t = sb1.tile([128, 6 * 128], F16)
        nc.sync.dma_start(gruW_t[:], gruW_d[:])
        gbias_t = sb1.tile([128, 4], F32)
        nc.sync.dma_start(gbias_t[:], gbias_d[:])
        bdeg_t = sb1.tile([128, NSH], F16)
        nc.sync.dma_start(bdeg_t[:], bdegT_d[:])
        ident_t = sb1.tile([128, 128], F16)
        nc.sync.dma_start(ident_t[:], ident_d[:])
        c1w_t = sb1.tile([128, 3 * 128], F16)
        nc.sync.dma_start(c1w_t[:], c1w_d[:])
        c2w_t = sb1.tile([128, 128], F16)
        nc.sync.dma_start(c2w_t[:], c2w_d[:])
        cc1w_t = sb1.tile([128, 12 * 128], F16)
        nc.sync.dma_start(cc1w_t[:], cc1w_d[:])
        cc2w_t = sb1.tile([128, 4 * 128], F16)
        nc.sync.dma_start(cc2w_t[:], cc2w_d[:])
        cbias_t = sb1.tile([128, 6], F32)
        nc.sync.dma_start(cbias_t[:], cbias_d[:])
        mlp_t = sb1.tile([128, 3], F16)
        nc.sync.dma_start(mlp_t[:], mlp_d[:])
        mlpb_t = sb1.tile([1, 2], F32)
        nc.sync.dma_start(mlpb_t[:], mlpb_d[:])

        hT_a = sb1.tile([128, NSH], F16)
        hT_b = sb1.tile([128, NSH], F16)
        hT0_t = sb1.tile([128, NSH], F16)
        nc.sync.dma_start(hT0_t[:], hT0_d[:])
        nc.sync.dma_start(hT_a[:], hT0_d[:])

        in_b = dram.tile([NSH, D], F16, name="in_b")
        A0 = dram.tile([AROWS, D], F16, name="A0")
        A1 = dram.tile([AROWS, D], F16, name="A1")
        A = (A0, A1)

        hT_cur, hT_nxt = hT_a, hT_b
        tab = None
        for s in range(STEPS * REPEAT):
            src_tab = h16_d if tab is None else tab
            # zero both A halves
            if not NOZERO:
                nc.sync.dma_start(A0[:], zeros_d[:])
                nc.sync.dma_start(A1[:], zeros_d[:])
            # gather all edges' source rows (wave order) in one call
            G = sb1.tile([128, TOT // 128, D], F16, tag="G", name="G")
            NR = TOT // 128
            if not NOGS:
                for r0 in range(0, NR, 64):
                    nr = min(64, NR - r0)
                    nc.gpsimd.dma_gather(
                        out_ap=G[:, r0:r0 + nr, :], in_ap=src_tab[:],
                        idxs_ap=gidx_t[:, r0 * 8:(r0 + nr) * 8],
                        num_idxs=nr * 128, num_idxs_reg=nr * 128, elem_size=D,
                        single_packet=False,
                    )
                # conflict-free scatter-add waves, <=32 rows per call
                for (hh, w0, wnr) in regions:
                    for r0 in range(w0, w0 + wnr, 32):
                        nr = min(32, w0 + wnr - r0)
                        nc.gpsimd.dma_scatter_add(
                            A[hh][:], G[:, r0:r0 + nr, :],
                            sidx_t[:, r0 * 8:(r0 + nr) * 8],
                            nr * 128, nr * 128, D,
                            single_packet=False,
                        )
            for hh in range(2):
                # A^T for this half via DMA transpose (DRAM -> SBUF)
                at = sb1.tile([128, HSLOT], F16, tag="at", name="at")
                nc.sync.dma_start(at[:], A[hh][0:HSLOT, :], transpose=True)
                at_v = at[:].rearrange("p (n t) -> p t n", t=T)
                HN = NSH // 2
                hh0 = hh * HN
                r_h = sbw.tile([128, HN], F16, tag="r_h", bufs=1)
                z_h = sbw.tile([128, HN], F16, tag="z_h", bufs=1)
                hn_h = sbw.tile([128, HN], F16, tag="hn_h", bufs=1)
                in_h = sbw.tile([128, HN], F16, tag="in_h", bufs=1)
                h_sl = hT_cur[:, hh0:hh0 + HN]
                # transform: 4 groups x 8 etypes into one 4-bank PSUM tile
                psA = ps.tile([128, HN], F32, space="PSUM", tag="psA", name="psA")
                for sgl in range(NSG // 2):
                    for t in range(T):
                        nc.tensor.matmul(
                            psA[:, sgl * SG:(sgl + 1) * SG],
                            lhsT=wmsg_t[:, t * 128:(t + 1) * 128],
                            rhs=at_v[:, t, sgl * SG:(sgl + 1) * SG],
                            start=(t == 0), stop=(t == T - 1),
                        )
                a_sb = sbw.tile([128, HN], F16, tag="a_sb", bufs=1)
                nc.vector.tensor_tensor(out=a_sb[:], in0=psA[:],
                                        in1=bdeg_t[:, hh0:hh0 + HN],
                                        op=ALU.add)
                # r gate (psG), z gate (psA), in gate (psG), hn gate (psA):
                # alternating buffers keep tensor engine ahead of activations
                psG = ps.tile([128, HN], F32, space="PSUM", tag="psG", name="psG")
                for sgl in range(NSG // 2):
                    sl = slice(sgl * SG, (sgl + 1) * SG)
                    nc.tensor.matmul(psG[:, sl], lhsT=gruW_t[:, 0:128],
                                     rhs=a_sb[:, sl], start=True, stop=False)
                    nc.tensor.matmul(psG[:, sl], lhsT=gruW_t[:, 3 * 128:4 * 128],
                                     rhs=h_sl[:, sl], start=False, stop=True)
                nc.scalar.activation(r_h[:], psG[:], AF.Sigmoid,
                                     bias=gbias_t[:, 0:1])
                psZ = ps.tile([128, HN], F32, space="PSUM", tag="psA", name="psZ")
                for sgl in range(NSG // 2):
                    sl = slice(sgl * SG, (sgl + 1) * SG)
                    nc.tensor.matmul(psZ[:, sl], lhsT=gruW_t[:, 128:256],
                                     rhs=a_sb[:, sl], start=True, stop=False)
                    nc.tensor.matmul(psZ[:, sl], lhsT=gruW_t[:, 4 * 128:5 * 128],
                                     rhs=h_sl[:, sl], start=False, stop=True)
                nc.scalar.activation(z_h[:], psZ[:], AF.Sigmoid,
                                     bias=gbias_t[:, 1:2])
                psI = ps.tile([128, HN], F32, space="PSUM", tag="psG", name="psI")
                for sgl in range(NSG // 2):
                    sl = slice(sgl * SG, (sgl + 1) * SG)
                    nc.tensor.matmul(psI[:, sl], lhsT=gruW_t[:, 2 * 128:3 * 128],
                                     rhs=a_sb[:, sl], start=True, stop=True)
                nc.scalar.copy(in_h[:], psI[:])
                psH = ps.tile([128, HN], F32, space="PSUM", tag="psA", name="psH")
                for sgl in range(NSG // 2):
                    sl = slice(sgl * SG, (sgl + 1) * SG)
                    nc.tensor.matmul(psH[:, sl], lhsT=gruW_t[:, 5 * 128:6 * 128],
                                     rhs=h_sl[:, sl], start=True, stop=True)
                nc.scalar.activation(hn_h[:], psH[:], AF.Identity,
                                     bias=gbias_t[:, 3:4])
                h_half = hT_cur[:, hh0:hh0 + HN]
                rhn_h = sbw.tile([128, HN], F16, tag="rhn_h", bufs=1, name="rhn_h")
                nc.vector.tensor_tensor(out=rhn_h[:], in0=r_h[:], in1=hn_h[:],
                                        op=ALU.mult)
                targ_h = sbw.tile([128, HN], F16, tag="targ_h", bufs=1, name="targ_h")
                nc.vector.tensor_tensor(out=targ_h[:], in0=in_h[:], in1=rhn_h[:],
                                        op=ALU.add)
                n_h = sbw.tile([128, HN], F16, tag="n_h", bufs=1, name="n_h")
                nc.scalar.activation(n_h[:], targ_h[:], AF.Tanh,
                                     bias=gbias_t[:, 2:3])
                d_h = sbw.tile([128, HN], F16, tag="rhn_h", bufs=1, name="d_h")
                nc.vector.tensor_tensor(out=d_h[:], in0=h_half, in1=n_h[:],
                                        op=ALU.subtract)
                zd_h = sbw.tile([128, HN], F16, tag="targ_h", bufs=1, name="zd_h")
                nc.vector.tensor_tensor(out=zd_h[:], in0=z_h[:], in1=d_h[:],
                                        op=ALU.mult)
                nc.vector.tensor_tensor(out=hT_nxt[:, hh0:hh0 + HN],
                                        in0=n_h[:], in1=zd_h[:], op=ALU.add)
                if s < STEPS * REPEAT - 1 and not NOROWS:
                    # h' rows for this half via SBUF->SBUF DMA transpose
                    rows = sbw.tile([128, 16, 128], F16, tag="rows", bufs=1)
                    nc.sync.dma_start(
                        rows[:], hT_nxt[:, hh * 2048:(hh + 1) * 2048],
                        transpose=True)
                    nc.sync.dma_start(
                        in_b[hh * 2048:(hh + 1) * 2048, :].rearrange(
                            "(j p) d -> p j d", p=128),
                        rows[:])

            if s < STEPS * REPEAT - 1 and not NOCOLL:
                tab = dram.tile([NN, D], F16, name=f"tab{s}")
                nc.gpsimd.collective_compute(
                    "AllGather", ALU.bypass,
                    replica_groups=[list(range(C))],
                    ins=[in_b[:].opt()], outs=[tab[:].opt()],
                )
            hT_cur, hT_nxt = hT_nxt, hT_cur

        nc.sync.dma_start(hdbg_d[:], hT_cur[:])

        # ------------------------------------------------------------ readout
        out_sb = sb1.tile([1, GPC], F32)
        if not READOUT:
            nc.gpsimd.memset(out_sb[:], 0.0)
        for g in range(GPC if READOUT else 0):
            goff = g * N
            y1 = sbw.tile([128, L1], F16, tag="y1", bufs=1)
            for p0, w in ((0, 512), (512, L1 - 512)):
                psy = ps.tile([128, 2048], F32, space="PSUM",
                              tag=("psA" if p0 == 0 else "psG"), name="psy")
                for k in range(3):
                    nc.tensor.matmul(psy[:, :w],
                                     lhsT=c1w_t[:, k * 128:(k + 1) * 128],
                                     rhs=hT_cur[:, goff + p0 + k:goff + p0 + k + w],
                                     start=(k == 0), stop=(k == 2))
                nc.scalar.activation(y1[:, p0:p0 + w], psy[:, :w], AF.Relu,
                                     bias=cbias_t[:, 0:1])
            y1e = y1[:, 0:1020].rearrange("p (l two) -> p two l", two=2)
            y1o = y1[:, 2:1022].rearrange("p (l two) -> p two l", two=2)
            m1_ = sbw.tile([128, P1], F16, tag="m1_", bufs=1)
            nc.vector.tensor_tensor(out=m1_[:], in0=y1e[:, 0, :], in1=y1e[:, 1, :],
                                    op=ALU.max)
            y1p = sbw.tile([128, P1], F16, tag="y1p", bufs=1)
            nc.vector.tensor_tensor(out=y1p[:], in0=m1_[:], in1=y1o[:, 0, :],
                                    op=ALU.max)
            psy2 = ps.tile([128, 2048], F32, space="PSUM", tag="psA", name="psy2")
            nc.tensor.matmul(psy2[:, :P1], lhsT=c2w_t[:], rhs=y1p[:],
                             start=True, stop=True)
            y2 = sbw.tile([128, P1], F16, tag="y2", bufs=1)
            nc.scalar.activation(y2[:], psy2[:, :P1], AF.Relu, bias=cbias_t[:, 1:2])
            y2v = y2[:, 0:510].rearrange("p (l two) -> p two l", two=2)
            y2p = sbw.tile([128, P2], F16, tag="y2p")
            nc.vector.tensor_tensor(out=y2p[:], in0=y2v[:, 0, :], in1=y2v[:, 1, :],
                                    op=ALU.max)
            psys = ps.tile([128, 2048], F32, space="PSUM", tag="psG", name="psys")
            nc.tensor.matmul(psys[0:1, :P2], lhsT=mlp_t[:, 0:1], rhs=y2p[:],
                             start=True, stop=True)
            yb = sbw.tile([1, P2], F32, tag="yb")
            nc.scalar.activation(yb[:], psys[0:1, :P2], AF.Identity,
                                 bias=mlpb_t[:, 0:1])
            zsrc = (hT_cur, hT0_t)
            z1 = [sbw.tile([128, L1], F16, tag=f"z1_{cb}", name=f"z1_{cb}",
                           bufs=1) for cb in range(2)]
            for cb in range(2):
                for p0, w in ((0, 512), (512, L1 - 512)):
                    psz = ps.tile([128, 2048], F32, space="PSUM",
                                  tag=("psA" if p0 == 0 else "psG"), name="psz")
                    first = True
                    for k in range(3):
                        for ci in range(2):
                            wofs = (k * 4 + ci * 2 + cb) * 128
                            nc.tensor.matmul(
                                psz[:, :w],
                                lhsT=cc1w_t[:, wofs:wofs + 128],
                                rhs=zsrc[ci][:, goff + p0 + k:goff + p0 + k + w],
                                start=first, stop=(k == 2 and ci == 1))
                            first = False
                    nc.scalar.activation(z1[cb][:, p0:p0 + w], psz[:, :w], AF.Relu,
                                         bias=cbias_t[:, 2 + cb:3 + cb])
            z1p = [sbw.tile([128, P1], F16, tag=f"z1p_{cb}", name=f"z1p_{cb}",
                            bufs=1) for cb in range(2)]
            for cb in range(2):
                z1e = z1[cb][:, 0:1020].rearrange("p (l two) -> p two l", two=2)
                z1o = z1[cb][:, 2:1022].rearrange("p (l two) -> p two l", two=2)
                mz = sbw.tile([128, P1], F16, tag="mz", bufs=1)
                nc.vector.tensor_tensor(out=mz[:], in0=z1e[:, 0, :],
                                        in1=z1e[:, 1, :], op=ALU.max)
                nc.vector.tensor_tensor(out=z1p[cb][:], in0=mz[:],
                                        in1=z1o[:, 0, :], op=ALU.max)
            z2p = [sbw.tile([128, P2], F16, tag=f"z2p_{cb}", name=f"z2p_{cb}")
                   for cb in range(2)]
            for cb in range(2):
                psz2 = ps.tile([128, 2048], F32, space="PSUM",
                               tag=("psA" if cb == 0 else "psG"), name="psz2")
                for ci in range(2):
                    nc.tensor.matmul(psz2[:, :P1],
                                     lhsT=cc2w_t[:, (ci * 2 + cb) * 128:
                                                 (ci * 2 + cb) * 128 + 128],
                                     rhs=z1p[ci][:],
                                     start=(ci == 0), stop=(ci == 1))
                z2 = sbw.tile([128, P1], F16, tag="z2", bufs=1)
                nc.scalar.activation(z2[:], psz2[:, :P1], AF.Relu,
                                     bias=cbias_t[:, 4 + cb:5 + cb])
                z2v = z2[:, 0:510].rearrange("p (l two) -> p two l", two=2)
                nc.vector.tensor_tensor(out=z2p[cb][:], in0=z2v[:, 0, :],
                                        in1=z2v[:, 1, :], op=ALU.max)
            pszs = ps.tile([128, 2048], F32, space="PSUM", tag="psA", name="pszs")
            for cb in range(2):
                nc.tensor.matmul(pszs[0:1, :P2], lhsT=mlp_t[:, 1 + cb:2 + cb],
                                 rhs=z2p[cb][:], start=(cb == 0), stop=(cb == 1))
            zb = sbw.tile([1, P2], F32, tag="zb")
            nc.scalar.activation(zb[:], pszs[0:1, :P2], AF.Identity,
                                 bias=mlpb_t[:, 1:2])
            prod = sbw.tile([1, P2], F32, tag="prod")
            nc.vector.tensor_tensor(out=prod[:], in0=yb[:], in1=zb[:], op=ALU.mult)
            red = sbw.tile([1, 1], F32, tag="red")
            nc.vector.reduce_sum(red[:], prod[:], axis=mybir.AxisListType.X)
            nc.scalar.activation(out_sb[:, g:g + 1], red[:], AF.Sigmoid,
                                 scale=1.0 / P2)
        nc.sync.dma_start(out_d[:], out_sb[:])

    nc.compile()
    return nc


# ------------------------------------------------------------------- wrapper
_CACHE = {}


def _host_tensors(inputs):
    W_msg = np.asarray(inputs["W_msg"], np.float32)
    b_msg = np.asarray(inputs["b_msg"], np.float32)
    w_ih = np.asarray(inputs["w_ih"], np.float32)
    w_hh = np.asarray(inputs["w_hh"], np.float32)
    b_ih = np.asarray(inputs["b_ih"], np.float32)
    b_hh = np.asarray(inputs["b_hh"], np.float32)
    wmsgT = np.concatenate([W_msg[t].T for t in range(T)], axis=1).astype(np.float16)
    gruW = np.concatenate(
        [w_ih[i * 128:(i + 1) * 128, :].T for i in range(3)]
        + [w_hh[i * 128:(i + 1) * 128, :].T for i in range(3)], axis=1
    ).astype(np.float16)
    gbias = np.stack([
        b_ih[0:128] + b_hh[0:128],
        b_ih[128:256] + b_hh[128:256],
        b_ih[256:384],
        b_hh[256:384],
    ], axis=1).astype(np.float32)
    c1w = np.asarray(inputs["conv1_w"], np.float32)
    c2w = np.asarray(inputs["conv2_w"], np.float32)
    cc1 = np.asarray(inputs["cc1_w"], np.float32)
    cc2 = np.asarray(inputs["cc2_w"], np.float32)
    c1wT = np.concatenate([c1w[:, :, k].T for k in range(3)], axis=1).astype(np.float16)
    c2wT = c2w[:, :, 0].T.astype(np.float16)
    cc1_blocks = []
    for k in range(3):
        for ci in range(2):
            for cb in range(2):
                cc1_blocks.append(
                    cc1[cb * 128:(cb + 1) * 128, ci * 128:(ci + 1) * 128, k].T)
    cc1wT = np.concatenate(cc1_blocks, axis=1).astype(np.float16)
    cc2_blocks = []
    for ci in range(2):
        for cb in range(2):
            cc2_blocks.append(cc2[cb * 128:(cb + 1) * 128,
                                  ci * 128:(ci + 1) * 128, 0].T)
    cc2wT = np.concatenate(cc2_blocks, axis=1).astype(np.float16)
    cbias = np.stack([
        np.asarray(inputs["conv1_b"], np.float32),
        np.asarray(inputs["conv2_b"], np.float32),
        np.asarray(inputs["cc1_b"], np.float32)[0:128],
        np.asarray(inputs["cc1_b"], np.float32)[128:256],
        np.asarray(inputs["cc2_b"], np.float32)[0:128],
        np.asarray(inputs["cc2_b"], np.float32)[128:256],
    ], axis=1).astype(np.float32)
    mlp_y_w = np.asarray(inputs["mlp_y_w"], np.float32)
    mlp_z_w = np.asarray(inputs["mlp_z_w"], np.float32)
    mlp = np.stack([mlp_y_w[0], mlp_z_w[0, 0:128], mlp_z_w[0, 128:256]],
                   axis=1).astype(np.float16)
    mlpb = np.array([[float(np.asarray(inputs["mlp_y_b"])[0]),
                      float(np.asarray(inputs["mlp_z_b"])[0])]], np.float32)
    return wmsgT, gruW, gbias, c1wT, c2wT, cc1wT, cc2wT, cbias, mlp, mlpb


def _prepare(**inputs):
    h = np.asarray(inputs["h"], np.float32)
    src = np.asarray(inputs["src"]); dst = np.asarray(inputs["dst"])
    etype = np.asarray(inputs["etype"])
    b_msg = np.asarray(inputs["b_msg"], np.float32)

    regions, TOT, gidx, sidx = _preprocess(src, dst, etype)
    key = (STEPS, READOUT, REPEAT, NOCOLL, NOGS, NOMM, NOZERO, NOTR, NOROWS,
           MMSG, TOT, tuple(regions))
    if key not in _CACHE:
        _CACHE[key] = _build(regions, TOT)
    nc = _CACHE[key]

    h16 = h.reshape(NN, D).astype(np.float16)
    biasdeg = np.zeros((NN, D), dtype=np.float32)
    np.add.at(biasdeg, dst, b_msg[etype])
    (wmsgT, gruW, gbias, c1wT, c2wT, cc1wT, cc2wT, cbias, mlp,
     mlpb) = _host_tensors(inputs)
    ident = np.eye(128, dtype=np.float16)
    zeros = np.zeros((AROWS, D), dtype=np.float16)

    in_maps = []
    for c in range(C):
        hT0 = np.ascontiguousarray(h16[c * NSH:(c + 1) * NSH].T)
        in_maps.append({
            "h16": h16, "hT0": hT0,
            "gidx": _wrap_idxs(gidx[c]), "sidx": _wrap_idxs(sidx[c]),
            "zeros": zeros,
            "wmsgT": wmsgT, "gruW": gruW, "gbias": gbias,
            "bdegT": np.ascontiguousarray(
                biasdeg[c * NSH:(c + 1) * NSH].T).astype(np.float16),
            "ident": ident,
            "c1w": c1wT, "c2w": c2wT, "cc1w": cc1wT, "cc2w": cc2wT,
            "cbias": cbias, "mlp": mlp, "mlpb": mlpb,
        })

    return nc, in_maps


def kernel(**inputs):
    nc, in_maps = _prepare(**inputs)
    res = run_bass_kernel_spmd(nc, in_maps, core_ids=list(range(C)))
    out = np.concatenate([res.results[c]["out"][0] for c in range(C)])
    kernel._last_results = res
    return out.astype(np.float32)


def make_runner(**inputs):
    """Persistent-jit executor for timing: jits the SPMD body once, keeps
    inputs device-resident, so per-call wall time ~= device exec time plus
    the backend's fixed launch overhead (which repetition-delta cancels)."""
    import jax
    from jax.sharding import Mesh, PartitionSpec, NamedSharding
    from jax.experimental.shard_map import shard_map
    from concourse import bass2jax

    nc, in_maps = _prepare(**inputs)
    n_cores = len(in_maps)
    bass2jax.install_neuronx_cc_hook()
    in_names, out_names, out_avals = [], [], []
    pname = nc.partition_id_tensor.name if nc.partition_id_tensor else None
    for alloc in nc.m.functions[0].allocations:
        if not isinstance(alloc, mybir.MemoryLocationSet):
            continue
        name = alloc.memorylocations[0].name
        if alloc.kind == "ExternalInput":
            if name != pname:
                in_names.append(name)
        elif alloc.kind == "ExternalOutput":
            out_names.append(name)
            out_avals.append(jax.core.ShapedArray(
                tuple(alloc.tensor_shape), mybir.dt.np(alloc.dtype)))
    n_params, n_outs = len(in_names), len(out_avals)
    all_in = in_names + out_names + ([pname] if pname else [])

    def _body(*args):
        operands = list(args)
        if pname is not None:
            operands.append(bass2jax.partition_id_tensor())
        return tuple(bass2jax._bass_exec_p.bind(
            *operands, out_avals=tuple(out_avals), in_names=tuple(all_in),
            out_names=tuple(out_names), lowering_input_output_aliases=(),
            sim_require_finite=True, sim_require_nnan=True, nc=nc))

    devices = jax.devices()[:n_cores]
    mesh = Mesh(np.asarray(devices), ("core",))
    sharded = jax.jit(
        shard_map(_body, mesh=mesh,
                  in_specs=(PartitionSpec("core"),) * (n_params + n_outs),
                  out_specs=(PartitionSpec("core"),) * n_outs,
                  check_rep=False),
        donate_argnums=tuple(range(n_params, n_params + n_outs)),
        keep_unused=True)
    sharding = NamedSharding(mesh, PartitionSpec("core"))
    dev_in = [jax.device_put(
        np.concatenate([np.asarray(in_maps[c][n]) for c in range(n_cores)],
                       axis=0), sharding) for n in in_names]
    zshapes = [(n_cores * av.shape[0], *av.shape[1:]) for av in out_avals]
    zdtypes = [av.dtype for av in out_avals]

    def run():
        zs = [jax.device_put(np.zeros(s, d), sharding)
              for s, d in zip(zshapes, zdtypes)]
        jax.block_until_ready(zs)
        outs = sharded(*dev_in, *zs)
        jax.block_until_ready(outs)
        return outs

    return run



# revision 29
# speedup vs baseline: 3.3527x; 3.3527x over previous
"""Instruction-minimal Trainium2 kernel for nn_DevignModel (v2).

Same math as kernel.py, but the per-(dst,etype)-slot aggregation is done with
hardware scatter-adds instead of one-hot matmuls:
  per step: 1 dma_gather (all edges' source rows, wave-sorted) ->
  16 dma_scatter_add waves (conflict-free within each wave; 2 halves so slot
  ids fit int16; padded lanes land in a trash slot) into A[slots,128] fp16 in
  DRAM -> 1 DMA-transpose per half to get A^T in SBUF -> per-etype transform
  matmuls + GRU (unchanged) -> h' rows via DMA-transpose -> AllGather.

This targets the measured ~60us/instruction dispatch floor of this system:
~230 instructions/step instead of ~640.
"""
import os
import sys
from contextlib import ExitStack

import numpy as np

sys.path.insert(0, "/opt/trn_rl_repo")
sys.path.insert(0, "/opt/trn_rl_repo/concourse")

import concourse.bacc as bacc
import concourse.bass as bass
import concourse.mybir as mybir
import concourse.tile as tile
from concourse.bass_utils import run_bass_kernel_spmd

F16 = mybir.dt.float16
F32 = mybir.dt.float32
I16 = mybir.dt.int16
AF = mybir.ActivationFunctionType
ALU = mybir.AluOpType

B, N, D = 32, 1024, 128
NN = B * N
E = 262144
T = 8
STEPS = int(os.environ.get("DEVIGN_STEPS", "6"))
READOUT = os.environ.get("DEVIGN_READOUT", "1") == "1"
REPEAT = int(os.environ.get("DEVIGN_REPEAT", "1"))
NOCOLL = os.environ.get("DEVIGN_NOCOLL", "0") == "1"
NOGS = os.environ.get("DEVIGN_NOGS", "0") == "1"      # skip gather+scatter (timing only)
NOMM = os.environ.get("DEVIGN_NOMM", "0") == "1"      # skip transform+GRU matmul block
NOZERO = os.environ.get("DEVIGN_NOZERO", "0") == "1"  # skip A zero-fill
NOTR = os.environ.get("DEVIGN_NOTR", "0") == "1"      # skip A^T DMA transposes
NOROWS = os.environ.get("DEVIGN_NOROWS", "0") == "1"  # skip h'-row transposes
MMSG = int(os.environ.get("DEVIGN_MMSG", "4"))        # sgl iters per half (4=full)

C = 8
NSH = NN // C            # 4096
SLOTS = NSH * T          # 32768
HSLOT = SLOTS // 2       # 16384 slots per half
AROWS = HSLOT + 128      # A rows per half (+ trash region)
TRASH = HSLOT            # trash slot id (local to half)
SG = 512
NSG = NSH // SG
GPC = NSH // N
L1 = N - 2
P1 = (L1 - 3) // 2 + 1
P2 = (P1 - 2) // 2 + 1


# ----------------------------------------------------------------- host prep
def _preprocess(src, dst, etype):
    """Wave-sorted edge lists. Returns (regions, gidx, sidx):
    regions: list of (half, row0, nrows) per scatter instruction, 128-row
             granular in the gather tile; shared across cores.
    gidx[c]: [TOT] int16 gather indices (global node ids; pads = 0)
    sidx[c]: [TOT] int16 scatter indices (half-local slot; pads = TRASH)
    """
    per_core = []
    for c in range(C):
        m = (dst // NSH) == c
        e_src = src[m].astype(np.int64)
        e_slot = (dst[m] - c * NSH).astype(np.int64) * T + etype[m]
        order = np.argsort(e_slot, kind="stable")
        es, sl = e_src[order], e_slot[order]
        first = np.searchsorted(sl, sl)
        rank = np.arange(sl.size) - first
        half = (sl >= HSLOT).astype(np.int64)
        assert rank.max() < 64
        # half-major wave order: all A0 waves first, so A0's transpose +
        # transform overlap with A1's scatters
        key = half * 64 + rank
        worder = np.lexsort((sl, key))
        per_core.append((es[worder], sl[worder], key[worder]))

    nkey = max(int(k[-1]) for _, _, k in per_core) + 1
    sizes = np.zeros(nkey, dtype=np.int64)
    for _, _, k in per_core:
        cnt = np.bincount(k, minlength=nkey)
        sizes = np.maximum(sizes, cnt)
    padded = ((sizes + 127) // 128) * 128

    regions = []
    row = 0
    for w in range(nkey):
        nr = int(padded[w]) // 128
        if nr == 0:
            continue
        regions.append((int(w // 64), row, nr))
        row += nr
    TOT = row * 128

    gidx = np.zeros((C, TOT), dtype=np.int16)
    sidx = np.full((C, TOT), TRASH, dtype=np.int16)
    for c, (es, sl, k) in enumerate(per_core):
        cnt = np.bincount(k, minlength=nkey)
        pos = 0
        row = 0
        for w in range(nkey):
            nr = int(padded[w]) // 128
            if nr == 0:
                continue
            n = int(cnt[w])
            o = row * 128
            gidx[c, o:o + n] = es[pos:pos + n]
            sidx[c, o:o + n] = (sl[pos:pos + n] % HSLOT)
            pos += n
            row += nr
    return regions, TOT, gidx, sidx


def _wrap_idxs(idx):
    n = idx.shape[0]
    w = idx.astype(np.int16).reshape(n // 16, 16).T
    return np.tile(w, (8, 1))


# --------------------------------------------------------------- device build
def _build(regions, TOT):
    nc = bacc.Bacc("TRN2", target_bir_lowering=False, debug=False, num_devices=C)

    h16_d = nc.dram_tensor("h16", [NN, D], F16, kind="ExternalInput")
    hT0_d = nc.dram_tensor("hT0", [128, NSH], F16, kind="ExternalInput")
    gidx_d = nc.dram_tensor("gidx", [128, TOT // 16], I16, kind="ExternalInput")
    sidx_d = nc.dram_tensor("sidx", [128, TOT // 16], I16, kind="ExternalInput")
    zeros_d = nc.dram_tensor("zeros", [AROWS, D], F16, kind="ExternalInput")
    wmsgT_d = nc.dram_tensor("wmsgT", [128, T * 128], F16, kind="ExternalInput")
    gruW_d = nc.dram_tensor("gruW", [128, 6 * 128], F16, kind="ExternalInput")
    gbias_d = nc.dram_tensor("gbias", [128, 4], F32, kind="ExternalInput")
    bdegT_d = nc.dram_tensor("bdegT", [128, NSH], F16, kind="ExternalInput")
    ident_d = nc.dram_tensor("ident", [128, 128], F16, kind="ExternalInput")
    c1w_d = nc.dram_tensor("c1w", [128, 3 * 128], F16, kind="ExternalInput")
    c2w_d = nc.dram_tensor("c2w", [128, 128], F16, kind="ExternalInput")
    cc1w_d = nc.dram_tensor("cc1w", [128, 12 * 128], F16, kind="ExternalInput")
    cc2w_d = nc.dram_tensor("cc2w", [128, 4 * 128], F16, kind="ExternalInput")
    cbias_d = nc.dram_tensor("cbias", [128, 6], F32, kind="ExternalInput")
    mlp_d = nc.dram_tensor("mlp", [128, 3], F16, kind="ExternalInput")
    mlpb_d = nc.dram_tensor("mlpb", [1, 2], F32, kind="ExternalInput")

    out_d = nc.dram_tensor("out", [1, GPC], F32, kind="ExternalOutput")
    hdbg_d = nc.dram_tensor("hdbg", [128, NSH], F16, kind="ExternalOutput")

    with tile.TileContext(nc) as tc, ExitStack() as ctx:
        sb1 = ctx.enter_context(tc.tile_pool(name="sb1", bufs=1))
        sbw = ctx.enter_context(tc.tile_pool(name="sbw", bufs=2))
        ps = ctx.enter_context(tc.tile_pool(name="ps", bufs=1, space="PSUM"))
        dram = ctx.enter_context(tc.tile_pool(name="dram", bufs=1, space="DRAM"))

        gidx_t = sb1.tile([128, TOT // 16], I16)
        nc.sync.dma_start(gidx_t[:], gidx_d[:])
        sidx_t = sb1.tile([128, TOT // 16], I16)
        nc.sync.dma_start(sidx_t[:], sidx_d[:])
        wmsg_t = sb1.tile([128, T * 128], F16)
        nc.sync.dma_start(wmsg_t[:], wmsgT_d[:])
        gruW_t = sb1.tile([128, 6 * 128], F16)
        nc.sync.dma_start(gruW_t[:], gruW_d[:])
        gbias_t = sb1.tile([128, 4], F32)
        nc.sync.dma_start(gbias_t[:], gbias_d[:])
        bdeg_t = sb1.tile([128, NSH], F16)
        nc.sync.dma_start(bdeg_t[:], bdegT_d[:])
        ident_t = sb1.tile([128, 128], F16)
        nc.sync.dma_start(ident_t[:], ident_d[:])
        c1w_t = sb1.tile([128, 3 * 128], F16)
        nc.sync.dma_start(c1w_t[:], c1w_d[:])
        c2w_t = sb1.tile([128, 128], F16)
        nc.sync.dma_start(c2w_t[:], c2w_d[:])
        cc1w_t = sb1.tile([128, 12 * 128], F16)
        nc.sync.dma_start(cc1w_t[:], cc1w_d[:])
        cc2w_t = sb1.tile([128, 4 * 128], F16)
        nc.sync.dma_start(cc2w_t[:], cc2w_d[:])
        cbias_t = sb1.tile([128, 6], F32)
        nc.sync.dma_start(cbias_t[:], cbias_d[:])
        mlp_t = sb1.tile([128, 3], F16)
        nc.sync.dma_start(mlp_t[:], mlp_d[:])
        mlpb_t = sb1.tile([1, 2], F32)
        nc.sync.dma_start(mlpb_t[:], mlpb_d[:])

        hT_a = sb1.tile([128, NSH], F16)
        hT_b = sb1.tile([128, NSH], F16)
        hT0_t = sb1.tile([128, NSH], F16)
        nc.sync.dma_start(hT0_t[:], hT0_d[:])
        nc.sync.dma_start(hT_a[:], hT0_d[:])

        in_b = dram.tile([NSH, D], F16, name="in_b")
        A0 = dram.tile([AROWS, D], F16, name="A0")
        A1 = dram.tile([AROWS, D], F16, name="A1")
        A = (A0, A1)

        hT_cur, hT_nxt = hT_a, hT_b
        tab = None
        for s in range(STEPS * REPEAT):
            src_tab = h16_d if tab is None else tab
            # zero both A halves
            if not NOZERO:
                nc.sync.dma_start(A0[:], zeros_d[:])
                nc.sync.dma_start(A1[:], zeros_d[:])
            # gather all edges' source rows (wave order) in one call
            G = sb1.tile([128, TOT // 128, D], F16, tag="G", name="G")
            NR = TOT // 128
            if not NOGS:
                for r0 in range(0, NR, 64):
                    nr = min(64, NR - r0)
                    nc.gpsimd.dma_gather(
                        out_ap=G[:, r0:r0 + nr, :], in_ap=src_tab[:],
                        idxs_ap=gidx_t[:, r0 * 8:(r0 + nr) * 8],
                        num_idxs=nr * 128, num_idxs_reg=nr * 128, elem_size=D,
                        single_packet=False,
                    )
                # conflict-free scatter-add waves, <=32 rows per call
                for (hh, w0, wnr) in regions:
                    for r0 in range(w0, w0 + wnr, 32):
                        nr = min(32, w0 + wnr - r0)
                        nc.gpsimd.dma_scatter_add(
                            A[hh][:], G[:, r0:r0 + nr, :],
                            sidx_t[:, r0 * 8:(r0 + nr) * 8],
                            nr * 128, nr * 128, D,
                            single_packet=False,
                        )
            for hh in range(2):
                # A^T for this half via DMA transpose (DRAM -> SBUF)
                at = sb1.tile([128, HSLOT], F16, tag="at", name="at")
                nc.sync.dma_start(at[:], A[hh][0:HSLOT, :], transpose=True)
                at_v = at[:].rearrange("p (n t) -> p t n", t=T)
                HN = NSH // 2
                hh0 = hh * HN
                r_h = sbw.tile([128, HN], F16, tag="r_h", bufs=1)
                z_h = sbw.tile([128, HN], F16, tag="z_h", bufs=1)
                hn_h = sbw.tile([128, HN], F16, tag="hn_h", bufs=1)
                in_h = sbw.tile([128, HN], F16, tag="in_h", bufs=1)
                h_sl = hT_cur[:, hh0:hh0 + HN]
                # transform: 4 groups x 8 etypes into one 4-bank PSUM tile
                psA = ps.tile([128, HN], F32, space="PSUM", tag="psA", name="psA")
                for sgl in range(NSG // 2):
                    for t in range(T):
                        nc.tensor.matmul(
                            psA[:, sgl * SG:(sgl + 1) * SG],
                            lhsT=wmsg_t[:, t * 128:(t + 1) * 128],
                            rhs=at_v[:, t, sgl * SG:(sgl + 1) * SG],
                            start=(t == 0), stop=(t == T - 1),
                        )
                a_sb = sbw.tile([128, HN], F16, tag="a_sb", bufs=1)
                nc.vector.tensor_tensor(out=a_sb[:], in0=psA[:],
                                        in1=bdeg_t[:, hh0:hh0 + HN],
                                        op=ALU.add)
                # r gate (psG), z gate (psA), in gate (psG), hn gate (psA):
                # alternating buffers keep tensor engine ahead of activations
                psG = ps.tile([128, HN], F32, space="PSUM", tag="psG", name="psG")
                for sgl in range(NSG // 2):
                    sl = slice(sgl * SG, (sgl + 1) * SG)
                    nc.tensor.matmul(psG[:, sl], lhsT=gruW_t[:, 0:128],
                                     rhs=a_sb[:, sl], start=True, stop=False)
                    nc.tensor.matmul(psG[:, sl], lhsT=gruW_t[:, 3 * 128:4 * 128],
                                     rhs=h_sl[:, sl], start=False, stop=True)
                nc.scalar.activation(r_h[:], psG[:], AF.Sigmoid,
                                     bias=gbias_t[:, 0:1])
                psZ = ps.tile([128, HN], F32, space="PSUM", tag="psA", name="psZ")
                for sgl in range(NSG // 2):
                    sl = slice(sgl * SG, (sgl + 1) * SG)
                    nc.tensor.matmul(psZ[:, sl], lhsT=gruW_t[:, 128:256],
                                     rhs=a_sb[:, sl], start=True, stop=False)
                    nc.tensor.matmul(psZ[:, sl], lhsT=gruW_t[:, 4 * 128:5 * 128],
                                     rhs=h_sl[:, sl], start=False, stop=True)
                nc.scalar.activation(z_h[:], psZ[:], AF.Sigmoid,
                                     bias=gbias_t[:, 1:2])
                psI = ps.tile([128, HN], F32, space="PSUM", tag="psG", name="psI")
                for sgl in range(NSG // 2):
                    sl = slice(sgl * SG, (sgl + 1) * SG)
                    nc.tensor.matmul(psI[:, sl], lhsT=gruW_t[:, 2 * 128:3 * 128],
                                     rhs=a_sb[:, sl], start=True, stop=True)
                nc.scalar.copy(in_h[:], psI[:])
                psH = ps.tile([128, HN], F32, space="PSUM", tag="psA", name="psH")
                for sgl in range(NSG // 2):
                    sl = slice(sgl * SG, (sgl + 1) * SG)
                    nc.tensor.matmul(psH[:, sl], lhsT=gruW_t[:, 5 * 128:6 * 128],
                                     rhs=h_sl[:, sl], start=True, stop=True)
                nc.scalar.activation(hn_h[:], psH[:], AF.Identity,
                                     bias=gbias_t[:, 3:4])
                h_half = hT_cur[:, hh0:hh0 + HN]
                rhn_h = sbw.tile([128, HN], F16, tag="rhn_h", bufs=1, name="rhn_h")
                nc.vector.tensor_tensor(out=rhn_h[:], in0=r_h[:], in1=hn_h[:],
                                        op=ALU.mult)
                targ_h = sbw.tile([128, HN], F16, tag="targ_h", bufs=1, name="targ_h")
                nc.vector.tensor_tensor(out=targ_h[:], in0=in_h[:], in1=rhn_h[:],
                                        op=ALU.add)
                n_h = sbw.tile([128, HN], F16, tag="n_h", bufs=1, name="n_h")
                nc.scalar.activation(n_h[:], targ_h[:], AF.Tanh,
                                     bias=gbias_t[:, 2:3])
                d_h = sbw.tile([128, HN], F16, tag="rhn_h", bufs=1, name="d_h")
                nc.vector.tensor_tensor(out=d_h[:], in0=h_half, in1=n_h[:],
                                        op=ALU.subtract)
                zd_h = sbw.tile([128, HN], F16, tag="targ_h", bufs=1, name="zd_h")
                nc.vector.tensor_tensor(out=zd_h[:], in0=z_h[:], in1=d_h[:],
                                        op=ALU.mult)
                nc.vector.tensor_tensor(out=hT_nxt[:, hh0:hh0 + HN],
                                        in0=n_h[:], in1=zd_h[:], op=ALU.add)
                if s < STEPS * REPEAT - 1 and not NOROWS:
                    # h' rows for this half via SBUF->SBUF DMA transpose
                    rows = sbw.tile([128, 16, 128], F16, tag="rows", bufs=1)
                    nc.sync.dma_start(
                        rows[:], hT_nxt[:, hh * 2048:(hh + 1) * 2048],
                        transpose=True)
                    nc.sync.dma_start(
                        in_b[hh * 2048:(hh + 1) * 2048, :].rearrange(
                            "(j p) d -> p j d", p=128),
                        rows[:])

            if s < STEPS * REPEAT - 1 and not NOCOLL:
                tab = dram.tile([NN, D], F16, name=f"tab{s}")
                nc.gpsimd.collective_compute(
                    "AllGather", ALU.bypass,
                    replica_groups=[list(range(C))],
                    ins=[in_b[:].opt()], outs=[tab[:].opt()],
                )
            hT_cur, hT_nxt = hT_nxt, hT_cur

        nc.sync.dma_start(hdbg_d[:], hT_cur[:])

        # ------------------------------------------------------------ readout
        out_sb = sb1.tile([1, GPC], F32)
        if not READOUT:
            nc.gpsimd.memset(out_sb[:], 0.0)
        for g in range(GPC if READOUT else 0):
            goff = g * N
            y1 = sbw.tile([128, L1], F16, tag="y1", bufs=1)
            for p0, w in ((0, 512), (512, L1 - 512)):
                psy = ps.tile([128, 2048], F32, space="PSUM",
                              tag=("psA" if p0 == 0 else "psG"), name="psy")
                for k in range(3):
                    nc.tensor.matmul(psy[:, :w],
                                     lhsT=c1w_t[:, k * 128:(k + 1) * 128],
                                     rhs=hT_cur[:, goff + p0 + k:goff + p0 + k + w],
                                     start=(k == 0), stop=(k == 2))
                nc.scalar.activation(y1[:, p0:p0 + w], psy[:, :w], AF.Relu,
                                     bias=cbias_t[:, 0:1])
            y1e = y1[:, 0:1020].rearrange("p (l two) -> p two l", two=2)
            y1o = y1[:, 2:1022].rearrange("p (l two) -> p two l", two=2)
            m1_ = sbw.tile([128, P1], F16, tag="m1_", bufs=1)
            nc.vector.tensor_tensor(out=m1_[:], in0=y1e[:, 0, :], in1=y1e[:, 1, :],
                                    op=ALU.max)
            y1p = sbw.tile([128, P1], F16, tag="y1p", bufs=1)
            nc.vector.tensor_tensor(out=y1p[:], in0=m1_[:], in1=y1o[:, 0, :],
                                    op=ALU.max)
            psy2 = ps.tile([128, 2048], F32, space="PSUM", tag="psA", name="psy2")
            nc.tensor.matmul(psy2[:, :P1], lhsT=c2w_t[:], rhs=y1p[:],
                             start=True, stop=True)
            y2 = sbw.tile([128, P1], F16, tag="y2", bufs=1)
            nc.scalar.activation(y2[:], psy2[:, :P1], AF.Relu, bias=cbias_t[:, 1:2])
            y2v = y2[:, 0:510].rearrange("p (l two) -> p two l", two=2)
            y2p = sbw.tile([128, P2], F16, tag="y2p")
            nc.vector.tensor_tensor(out=y2p[:], in0=y2v[:, 0, :], in1=y2v[:, 1, :],
                                    op=ALU.max)
            psys = ps.tile([128, 2048], F32, space="PSUM", tag="psG", name="psys")
            nc.tensor.matmul(psys[0:1, :P2], lhsT=mlp_t[:, 0:1], rhs=y2p[:],
                             start=True, stop=True)
            yb = sbw.tile([1, P2], F32, tag="yb")
            nc.scalar.activation(yb[:], psys[0:1, :P2], AF.Identity,
                                 bias=mlpb_t[:, 0:1])
            zsrc = (hT_cur, hT0_t)
            z1 = [sbw.tile([128, L1], F16, tag=f"z1_{cb}", name=f"z1_{cb}",
                           bufs=1) for cb in range(2)]
            for cb in range(2):
                for p0, w in ((0, 512), (512, L1 - 512)):
                    psz = ps.tile([128, 2048], F32, space="PSUM",
                                  tag=("psA" if p0 == 0 else "psG"), name="psz")
                    first = True
                    for k in range(3):
                        for ci in range(2):
                            wofs = (k * 4 + ci * 2 + cb) * 128
                            nc.tensor.matmul(
                                psz[:, :w],
                                lhsT=cc1w_t[:, wofs:wofs + 128],
                                rhs=zsrc[ci][:, goff + p0 + k:goff + p0 + k + w],
                                start=first, stop=(k == 2 and ci == 1))
                            first = False
                    nc.scalar.activation(z1[cb][:, p0:p0 + w], psz[:, :w], AF.Relu,
                                         bias=cbias_t[:, 2 + cb:3 + cb])
            z1p = [sbw.tile([128, P1], F16, tag=f"z1p_{cb}", name=f"z1p_{cb}",
                            bufs=1) for cb in range(2)]
            for cb in range(2):
                z1e = z1[cb][:, 0:1020].rearrange("p (l two) -> p two l", two=2)
                z1o = z1[cb][:, 2:1022].rearrange("p (l two) -> p two l", two=2)
                mz = sbw.tile([128, P1], F16, tag="mz", bufs=1)
                nc.vector.tensor_tensor(out=mz[:], in0=z1e[:, 0, :],
                                        in1=z1e[:, 1, :], op=ALU.max)
                nc.vector.tensor_tensor(out=z1p[cb][:], in0=mz[:],
                                        in1=z1o[:, 0, :], op=ALU.max)
            z2p = [sbw.tile([128, P2], F16, tag=f"z2p_{cb}", name=f"z2p_{cb}")
                   for cb in range(2)]
            for cb in range(2):
                psz2 = ps.tile([128, 2048], F32, space="PSUM",
                               tag=("psA" if cb == 0 else "psG"), name="psz2")
                for ci in range(2):
                    nc.tensor.matmul(psz2[:, :P1],
                                     lhsT=cc2w_t[:, (ci * 2 + cb) * 128:
                                                 (ci * 2 + cb) * 128 + 128],
                                     rhs=z1p[ci][:],
                                     start=(ci == 0), stop=(ci == 1))
                z2 = sbw.tile([128, P1], F16, tag="z2", bufs=1)
                nc.scalar.activation(z2[:], psz2[:, :P1], AF.Relu,
                                     bias=cbias_t[:, 4 + cb:5 + cb])
                z2v = z2[:, 0:510].rearrange("p (l two) -> p two l", two=2)
                nc.vector.tensor_tensor(out=z2p[cb][:], in0=z2v[:, 0, :],
                                        in1=z2v[:, 1, :], op=ALU.max)
            pszs = ps.tile([128, 2048], F32, space="PSUM", tag="psA", name="pszs")
            for cb in range(2):
                nc.tensor.matmul(pszs[0:1, :P2], lhsT=mlp_t[:, 1 + cb:2 + cb],
                                 rhs=z2p[cb][:], start=(cb == 0), stop=(cb == 1))
            zb = sbw.tile([1, P2], F32, tag="zb")
            nc.scalar.activation(zb[:], pszs[0:1, :P2], AF.Identity,
                                 bias=mlpb_t[:, 1:2])
            prod = sbw.tile([1, P2], F32, tag="prod")
            nc.vector.tensor_tensor(out=prod[:], in0=yb[:], in1=zb[:], op=ALU.mult)
            red = sbw.tile([1, 1], F32, tag="red")
            nc.vector.reduce_sum(red[:], prod[:], axis=mybir.AxisListType.X)
            nc.scalar.activation(out_sb[:, g:g + 1], red[:], AF.Sigmoid,
                                 scale=1.0 / P2)
        nc.sync.dma_start(out_d[:], out_sb[:])

    nc.compile()
    return nc


# ------------------------------------------------------------------- wrapper
_CACHE = {}


def _host_tensors(inputs):
    W_msg = np.asarray(inputs["W_msg"], np.float32)
    b_msg = np.asarray(inputs["b_msg"], np.float32)
    w_ih = np.asarray(inputs["w_ih"], np.float32)
    w_hh = np.asarray(inputs["w_hh"], np.float32)
    b_ih = np.asarray(inputs["b_ih"], np.float32)
    b_hh = np.asarray(inputs["b_hh"], np.float32)
    wmsgT = np.concatenate([W_msg[t].T for t in range(T)], axis=1).astype(np.float16)
    gruW = np.concatenate(
        [w_ih[i * 128:(i + 1) * 128, :].T for i in range(3)]
        + [w_hh[i * 128:(i + 1) * 128, :].T for i in range(3)], axis=1
    ).astype(np.float16)
    gbias = np.stack([
        b_ih[0:128] + b_hh[0:128],
        b_ih[128:256] + b_hh[128:256],
        b_ih[256:384],
        b_hh[256:384],
    ], axis=1).astype(np.float32)
    c1w = np.asarray(inputs["conv1_w"], np.float32)
    c2w = np.asarray(inputs["conv2_w"], np.float32)
    cc1 = np.asarray(inputs["cc1_w"], np.float32)
    cc2 = np.asarray(inputs["cc2_w"], np.float32)
    c1wT = np.concatenate([c1w[:, :, k].T for k in range(3)], axis=1).astype(np.float16)
    c2wT = c2w[:, :, 0].T.astype(np.float16)
    cc1_blocks = []
    for k in range(3):
        for ci in range(2):
            for cb in range(2):
                cc1_blocks.append(
                    cc1[cb * 128:(cb + 1) * 128, ci * 128:(ci + 1) * 128, k].T)
    cc1wT = np.concatenate(cc1_blocks, axis=1).astype(np.float16)
    cc2_blocks = []
    for ci in range(2):
        for cb in range(2):
            cc2_blocks.append(cc2[cb * 128:(cb + 1) * 128,
                                  ci * 128:(ci + 1) * 128, 0].T)
    cc2wT = np.concatenate(cc2_blocks, axis=1).astype(np.float16)
    cbias = np.stack([
        np.asarray(inputs["conv1_b"], np.float32),
        np.asarray(inputs["conv2_b"], np.float32),
        np.asarray(inputs["cc1_b"], np.float32)[0:128],
        np.asarray(inputs["cc1_b"], np.float32)[128:256],
        np.asarray(inputs["cc2_b"], np.float32)[0:128],
        np.asarray(inputs["cc2_b"], np.float32)[128:256],
    ], axis=1).astype(np.float32)
    mlp_y_w = np.asarray(inputs["mlp_y_w"], np.float32)
    mlp_z_w = np.asarray(inputs["mlp_z_w"], np.float32)
    mlp = np.stack([mlp_y_w[0], mlp_z_w[0, 0:128], mlp_z_w[0, 128:256]],
                   axis=1).astype(np.float16)
    mlpb = np.array([[float(np.asarray(inputs["mlp_y_b"])[0]),
                      float(np.asarray(inputs["mlp_z_b"])[0])]], np.float32)
    return wmsgT, gruW, gbias, c1wT, c2wT, cc1wT, cc2wT, cbias, mlp, mlpb


def _prepare(**inputs):
    h = np.asarray(inputs["h"], np.float32)
    src = np.asarray(inputs["src"]); dst = np.asarray(inputs["dst"])
    etype = np.asarray(inputs["etype"])
    b_msg = np.asarray(inputs["b_msg"], np.float32)

    regions, TOT, gidx, sidx = _preprocess(src, dst, etype)
    key = (STEPS, READOUT, REPEAT, NOCOLL, NOGS, NOMM, NOZERO, NOTR, NOROWS,
           MMSG, TOT, tuple(regions))
    if key not in _CACHE:
        _CACHE[key] = _build(regions, TOT)
    nc = _CACHE[key]

    h16 = h.reshape(NN, D).astype(np.float16)
    biasdeg = np.zeros((NN, D), dtype=np.float32)
    np.add.at(biasdeg, dst, b_msg[etype])
    (wmsgT, gruW, gbias, c1wT, c2wT, cc1wT, cc2wT, cbias, mlp,
     mlpb) = _host_tensors(inputs)
    ident = np.eye(128, dtype=np.float16)
    zeros = np.zeros((AROWS, D), dtype=np.float16)

    in_maps = []
    for c in range(C):
        hT0 = np.ascontiguousarray(h16[c * NSH:(c + 1) * NSH].T)
        in_maps.append({
            "h16": h16, "hT0": hT0,
            "gidx": _wrap_idxs(gidx[c]), "sidx": _wrap_idxs(sidx[c]),
            "zeros": zeros,
            "wmsgT": wmsgT, "gruW": gruW, "gbias": gbias,
            "bdegT": np.ascontiguousarray(
                biasdeg[c * NSH:(c + 1) * NSH].T).astype(np.float16),
            "ident": ident,
            "c1w": c1wT, "c2w": c2wT, "cc1w": cc1wT, "cc2w": cc2wT,
            "cbias": cbias, "mlp": mlp, "mlpb": mlpb,
        })

    return nc, in_maps


def kernel(**inputs):
    nc, in_maps = _prepare(**inputs)
    res = run_bass_kernel_spmd(nc, in_maps, core_ids=list(range(C)))
    out = np.concatenate([res.results[c]["out"][0] for c in range(C)])
    kernel._last_results = res
    return out.astype(np.float32)


def make_runner(**inputs):
    """Persistent-jit executor for timing: jits the SPMD body once, keeps
    inputs device-resident, so per-call wall time ~= device exec time plus
    the backend's fixed launch overhead (which repetition-delta cancels)."""
    import jax
    from jax.sharding import Mesh, PartitionSpec, NamedSharding
    from jax.experimental.shard_map import shard_map
    from concourse import bass2jax

    nc, in_maps = _prepare(**inputs)
    n_cores = len(in_maps)
    bass2jax.install_neuronx_cc_hook()
    in_names, out_names, out_avals = [], [], []
    pname = nc.partition_id_tensor.name if nc.partition_id_tensor else None
    for alloc in nc.m.functions[0].allocations:
        if not isinstance(alloc, mybir.MemoryLocationSet):
            continue
        name = alloc.memorylocations[0].name
        if alloc.kind == "ExternalInput":
            if name != pname:
                in_names.append(name)
        elif alloc.kind == "ExternalOutput":
            out_names.append(name)
            out_avals.append(jax.core.ShapedArray(
                tuple(alloc.tensor_shape), mybir.dt.np(alloc.dtype)))
    n_params, n_outs = len(in_names), len(out_avals)
    all_in = in_names + out_names + ([pname] if pname else [])

    def _body(*args):
        operands = list(args)
        if pname is not None:
            operands.append(bass2jax.partition_id_tensor())
        return tuple(bass2jax._bass_exec_p.bind(
            *operands, out_avals=tuple(out_avals), in_names=tuple(all_in),
            out_names=tuple(out_names), lowering_input_output_aliases=(),
            sim_require_finite=True, sim_require_nnan=True, nc=nc))

    devices = jax.devices()[:n_cores]
    mesh = Mesh(np.asarray(devices), ("core",))
    sharded = jax.jit(
        shard_map(_body, mesh=mesh,
                  in_specs=(PartitionSpec("core"),) * (n_params + n_outs),
                  out_specs=(PartitionSpec("core"),) * n_outs,
                  check_rep=False),
        donate_argnums=tuple(range(n_params, n_params + n_outs)),
        keep_unused=True)
    sharding = NamedSharding(mesh, PartitionSpec("core"))
    dev_in = [jax.device_put(
        np.concatenate([np.asarray(in_maps[c][n]) for c in range(n_cores)],
                       axis=0), sharding) for n in in_names]
    zshapes = [(n_cores * av.shape[0], *av.shape[1:]) for av in out_avals]
    zdtypes = [av.dtype for av in out_avals]

    def run():
        zs = [jax.device_put(np.zeros(s, d), sharding)
              for s, d in zip(zshapes, zdtypes)]
        jax.block_until_ready(zs)
        outs = sharded(*dev_in, *zs)
        jax.block_until_ready(outs)
        return outs

    run.out_names = tuple(out_names)
    run.n_cores = n_cores
    return run

